# revision 2
# baseline (speedup 1.0000x reference)
"""Trainium2 Bass kernel v3 for nn_BaseModel_32255204393001.

Sharding (8 cores): batch 256 -> 2 groups of 128 (cores 0-3 / 4-7).
Asymmetric GRU split: L0 computed FULL (replicated per core, batch 128) --
its ~14us of PE work fills the AllGather window and keeps the PE warm; L1
gate-sharded 4-way; ONE AllGather per step carrying only h1 slices
([128,256] bf16, out 256KB). 120 encode + 24 rollout AGs total.
Rollout: L0-full + L1-sharded + AG(h1) + replicated spline (no x gather).
Attention in bf16 (fp32 softmax internals). bf16 matmuls, fp32 state/PSUM.
"""

import numpy as np
import ml_dtypes

import concourse.bacc as bacc
import concourse.mybir as mybir
import concourse.tile as tile
from concourse.masks import make_identity

F32 = mybir.dt.float32
BF16 = mybir.dt.bfloat16
AF = mybir.ActivationFunctionType
ALU = mybir.AluOpType

B, T, PRED, J, H, D = 256, 120, 24, 15, 1024, 135
HEADS = 4
FQ = T // 2 + 1          # 61 freq bins
BC = 128                 # batch per group
GSL = 768                # L1 gate slice per lane (r|z|n 256 each)
CA = 16                  # attention batch sub-chunk (freq stage)
CB = 32                  # attention batch chunk
GROUPS = [[0, 1, 2, 3], [4, 5, 6, 7]]

T_STEPS = T
PRED_STEPS = PRED


def _bf(x):
    return np.asarray(x, dtype=ml_dtypes.bfloat16)


def build_module(t_steps=T_STEPS, pred_steps=PRED_STEPS):
    nc = bacc.Bacc("TRN2", target_bir_lowering=False, debug=False, num_devices=8)

    def din(name, shape, dt=BF16):
        return nc.dram_tensor(name, shape, dt, kind="ExternalInput")

    # ---- attention inputs (fp32, per-lane head slice) ----
    poses = din("poses_b", [BC, T, D])
    wf_qk32 = din("wf_qk32", [D, 768], F32)
    poses32 = din("poses32", [BC, T, D], F32)
    ct32 = din("ct32", [T, FQ], F32)
    bf_qk32 = din("bf_qk32", [1, 768], F32)
    wout_h = din("wout_h", [256, H], F32)
    aob4 = din("aob4", [1, H], F32)
    # ---- L0 full weights (natural gate order r|z|n) ----
    wih0f = din("wih0f", [D, 3 * H])
    whh0f = din("whh0f", [H, 3 * H])
    b0rz = din("b0rz", [1, 2 * H]); b0ni = din("b0ni", [1, H]); b0nh = din("b0nh", [1, H])
    # ---- L1 sharded weights (per-lane gate slices r|z|n 256 each) ----
    wih1T = din("wih1T_s", [H, GSL])
    whh1T = din("whh1T_s", [H, GSL])
    brz1 = din("brz1", [1, 512]); bni1 = din("bni1", [1, 256]); bnh1 = din("bnh1", [1, 256])
    # ---- rollout spline weights (replicated) ----
    pre_wT = din("pre_wT", [H, H])
    pre_b = din("pre_b_row", [1, H])
    spl1T = din("spl1T_f", [H, J * 128])
    spl1b = din("spl1b_f", [1, J * 128])
    spl2 = din("spl2_f", [128, J * 6])
    spl2b = din("spl2b_f", [1, J * 6])
    # ---- rollout init (group batch 128) ----
    x0T = din("x0T", [D, BC])
    prev6d0 = din("prev6d0_s", [BC, J * 6], F32)

    out6d = nc.dram_tensor("out6d", [PRED, BC, J * 6], F32, kind="ExternalOutput")

    TS = min(t_steps, T)

    with tile.TileContext(nc) as tc:
        # ---------------- rs: whole-kernel constants ----------------
        rs_cm = tc.tile_pool(name="rs", bufs=1)
        rs = rs_cm.__enter__()
        ident = rs.tile([128, 128], BF16)
        make_identity(nc, ident)
        ones_col = rs.tile([1, 128], BF16)
        nc.vector.memset(ones_col[:], 1.0)
        ctx_sb = rs.tile([BC, H], F32, tag="ctx_sb")

        # ---------------- wp: weights + persistent state ----------------
        wp_cm = tc.tile_pool(name="wp", bufs=1)
        wp = wp_cm.__enter__()
        ones32 = wp.tile([1, 512], F32, tag="ones32")
        nc.vector.memset(ones32[:], 1.0)
        ones61_32 = wp.tile([FQ, 1], F32, tag="ones61_32")
        nc.vector.memset(ones61_32[:], 1.0 / FQ)

        def load(t_dram, dt=BF16, pool=None, tag=None):
            p = pool or wp
            tl = p.tile(list(t_dram.shape), dt, tag=tag or t_dram.name)
            nc.sync.dma_start(out=tl[:], in_=t_dram[:])
            return tl

        wih0hi = wp.tile([128, 3 * H], BF16, tag="wih0hi")
        nc.sync.dma_start(out=wih0hi[:], in_=wih0f[0:128, :])
        wih0lo = wp.tile([7, 3 * H], BF16, tag="wih0lo")
        nc.sync.dma_start(out=wih0lo[:], in_=wih0f[128:D, :])
        b0rz_sb = load(b0rz); b0ni_sb = load(b0ni); b0nh_sb = load(b0nh)
        brz1_sb = load(brz1); bni1_sb = load(bni1); bnh1_sb = load(bnh1)

        # state: h0 full (replicated), h1 own lane slice
        h0f = wp.tile([BC, H], F32, tag="h0f"); nc.vector.memset(h0f[:], 0.0)
        h0T = wp.tile([128, H], BF16, tag="h0T"); nc.vector.memset(h0T[:], 0.0)
        h1 = wp.tile([BC, 256], F32, tag="h1"); nc.vector.memset(h1[:], 0.0)
        h1T_hold = wp.tile([128, H], BF16, tag="h1T_hold")

        # ---------------- phase B: attention ----------------
        with tc.tile_pool(name="pb", bufs=1) as pb, \
             tc.tile_pool(name="pb2", bufs=1) as pb2, \
             tc.tile_pool(name="ps1", bufs=1, space="PSUM") as ps1:
            ct32_sb = load(ct32, dt=F32, pool=pb)
            wfq32hi = pb.tile([128, 768], F32, tag="wfq32hi")
            nc.sync.dma_start(out=wfq32hi[:], in_=wf_qk32[0:128, :])
            wfq32lo = pb.tile([7, 768], F32, tag="wfq32lo")
            nc.sync.dma_start(out=wfq32lo[:], in_=wf_qk32[128:D, :])
            bfqk32_sb = load(bf_qk32, dt=F32, pool=pb)
            wout0 = pb.tile([128, H], F32, tag="wout0")
            nc.sync.dma_start(out=wout0[:], in_=wout_h[0:128, :])
            wout1 = pb.tile([128, H], F32, tag="wout1")
            nc.sync.dma_start(out=wout1[:], in_=wout_h[128:256, :])
            aob_sb = load(aob4, dt=F32, pool=pb)

            O_hi = pb.tile([128, BC], F32, tag="O_hi")
            O_lo = pb.tile([128, BC], F32, tag="O_lo")
            for ci in range(BC // CB):
                fq32hi = pb2.tile([128, FQ * CB], F32, tag="fq32hi")
                fq32lo = pb2.tile([7, FQ * CB], F32, tag="fq32lo")
                XC32 = pb2.tile([T, CB * D], F32, tag="XC32")
                nc.sync.dma_start(out=XC32[:].rearrange("p (b d) -> p b d", d=D),
                                  in_=poses32[ci * CB:(ci + 1) * CB, :, :].rearrange("b t d -> t b d"))
                for bi in range(CB):
                    pdd = ps1.tile([128, 128], F32, tag="pdd")
                    nc.tensor.matmul(pdd[:, 0:FQ], XC32[:, bi * D: bi * D + 128], ct32_sb[:], start=True, stop=True)
                    nc.tensor.matmul(pdd[0:7, 64:64 + FQ], XC32[:, bi * D + 128: (bi + 1) * D], ct32_sb[:], start=True, stop=True)
                    nc.vector.tensor_copy(fq32hi[:, bi * FQ:(bi + 1) * FQ], pdd[:, 0:FQ])
                    nc.vector.tensor_copy(fq32lo[:, bi * FQ:(bi + 1) * FQ], pdd[0:7, 64:64 + FQ])
                qk = [pb2.tile([128, FQ * CB], F32, tag=f"qk{m}", name=f"qk{m}") for m in range(4)]
                NCB = FQ * CB  # 1952
                for m in range(4):
                    for s in range(4):
                        c0 = s * 488
                        c1 = min(c0 + 488, NCB)
                        pq = ps1.tile([128, 488], F32, tag="big")
                        nc.tensor.matmul(pq[:, 0:c1 - c0], wfq32hi[:, m * 128:(m + 1) * 128], fq32hi[:, c0:c1], start=True, stop=False)
                        nc.tensor.matmul(pq[:, 0:c1 - c0], wfq32lo[:, m * 128:(m + 1) * 128], fq32lo[:, c0:c1], start=False, stop=False)
                        nc.tensor.matmul(pq[:, 0:c1 - c0], bfqk32_sb[:, m * 128:(m + 1) * 128], ones32[:, 0:c1 - c0], start=False, stop=True)
                        nc.vector.tensor_copy(qk[m][:, c0:c1], pq[:, 0:c1 - c0])
                S_sb = pb2.tile([FQ, FQ * CB], F32, tag="S_sb")
                for bg in range(CB // 8):
                    pS = ps1.tile([FQ, 8 * FQ], F32, tag="S")
                    for bi8 in range(8):
                        bi = bg * 8 + bi8
                        sl = slice(bi * FQ, (bi + 1) * FQ)
                        psl = slice(bi8 * FQ, (bi8 + 1) * FQ)
                        nc.tensor.matmul(pS[:, psl], qk[0][:, sl], qk[2][:, sl], start=True, stop=False)
                        nc.tensor.matmul(pS[:, psl], qk[1][:, sl], qk[3][:, sl], start=False, stop=True)
                    nc.vector.tensor_copy(S_sb[:, bg * 8 * FQ:(bg + 1) * 8 * FQ], pS[:])
                S3 = S_sb[:].rearrange("p (b k) -> p b k", k=FQ)
                mx = pb.tile([FQ, CB], F32, tag="mx")
                nc.vector.reduce_max(mx[:, :, None], S3, axis=mybir.AxisListType.X)
                nc.vector.tensor_tensor(out=S3, in0=S3,
                                        in1=mx[:, :, None].broadcast_to([FQ, CB, FQ]),
                                        op=ALU.subtract)
                nc.scalar.activation(S_sb[:], S_sb[:], AF.Exp, scale=1.0 / 16.0)
                sm = pb.tile([FQ, CB], F32, tag="sm")
                nc.vector.reduce_sum(sm[:, :, None], S3, axis=mybir.AxisListType.X)
                rs = pb.tile([FQ, CB], F32, tag="rs")
                nc.vector.reciprocal(rs[:], sm[:])
                Wt = pb2.tile([FQ, FQ * CB], F32, tag="Wt")
                nc.vector.tensor_tensor(out=Wt[:].rearrange("p (b k) -> p b k", k=FQ),
                                        in0=S3,
                                        in1=rs[:, :, None].broadcast_to([FQ, CB, FQ]), op=ALU.mult)
                pam = ps1.tile([FQ, CB], F32, tag="am")
                for bi in range(CB):
                    nc.tensor.matmul(pam[:, bi:bi + 1], Wt[:, bi * FQ:(bi + 1) * FQ], ones61_32[:],
                                     start=True, stop=True)
                A_sb = pb.tile([FQ, CB], F32, tag="A_sb")
                nc.vector.tensor_copy(A_sb[:], pam[:])
                pOh = ps1.tile([128, CB], F32, tag="pOh")
                pOl = ps1.tile([128, CB], F32, tag="pOl")
                for bi in range(CB):
                    sl = slice(bi * FQ, (bi + 1) * FQ)
                    pv = ps1.tile([FQ, 256], F32, tag="pv")
                    nc.tensor.matmul(pv[:], fq32hi[:, sl], wfq32hi[:, 512:768], start=True, stop=False)
                    nc.tensor.matmul(pv[:], fq32lo[:, sl], wfq32lo[:, 512:768], start=False, stop=False)
                    nc.tensor.matmul(pv[:], ones32[:, 0:FQ], bfqk32_sb[:, 512:768], start=False, stop=True)
                    v_sb = pb2.tile([FQ, 256], F32, tag="v_sb")
                    nc.vector.tensor_copy(v_sb[:], pv[:])
                    nc.tensor.matmul(pOh[:, bi:bi + 1], v_sb[:, 0:128], A_sb[:, bi:bi + 1],
                                     start=True, stop=True)
                    nc.tensor.matmul(pOl[:, bi:bi + 1], v_sb[:, 128:256], A_sb[:, bi:bi + 1],
                                     start=True, stop=True)
                nc.vector.tensor_copy(O_hi[:, ci * CB:(ci + 1) * CB], pOh[:])
                nc.vector.tensor_copy(O_lo[:, ci * CB:(ci + 1) * CB], pOl[:])
            arin = pb.tile([BC, H], F32, tag="arin")
            for s in range(2):
                sl = slice(s * 512, (s + 1) * 512)
                pc = ps1.tile([BC, 512], F32, tag="big")
                nc.tensor.matmul(pc[:], O_hi[:], wout0[:, sl], start=True, stop=False)
                nc.tensor.matmul(pc[:], O_lo[:], wout1[:, sl], start=False, stop=False)
                nc.tensor.matmul(pc[:], ones32[:, 0:128], aob_sb[:, sl], start=False, stop=True)
                nc.vector.tensor_copy(arin[:, sl], pc[:])
            with tc.tile_pool(name="drA", bufs=1, space="DRAM") as drA:
                ar_in = drA.tile([BC, H], F32)
                ar_out = drA.tile([BC, H], F32)
                nc.sync.dma_start(out=ar_in[:], in_=arin[:])
                nc.gpsimd.collective_compute("AllReduce", ALU.add, ins=[ar_in.opt()],
                                             outs=[ar_out.opt()], replica_groups=GROUPS)
                nc.sync.dma_start(out=ctx_sb[:], in_=ar_out[:])

        # ---------------- wq: big GRU weights (after attention frees SBUF) ----
        wq_cm = tc.tile_pool(name="wq", bufs=1)
        wq = wq_cm.__enter__()
        whh0_k = [wq.tile([128, 3 * H], BF16, tag=f"whh0_{k}", name=f"whh0_{k}") for k in range(8)]
        wih1_k = [wq.tile([128, GSL], BF16, tag=f"wih1_{k}", name=f"wih1_{k}") for k in range(8)]
        whh1_k = [wq.tile([128, GSL], BF16, tag=f"whh1_{k}", name=f"whh1_{k}") for k in range(8)]
        for k in range(8):
            nc.sync.dma_start(out=whh0_k[k][:], in_=whh0f[k * 128:(k + 1) * 128, :])
            nc.sync.dma_start(out=wih1_k[k][:], in_=wih1T[k * 128:(k + 1) * 128, :])
            nc.sync.dma_start(out=whh1_k[k][:], in_=whh1T[k * 128:(k + 1) * 128, :])

        # ---------------- encode-only pools ----------------
        XP_cm = tc.tile_pool(name="XP", bufs=1)
        XP = XP_cm.__enter__()
        XThi = XP.tile([128, TS * 128], BF16, tag="XThi")
        XTlo_dr_cm = tc.tile_pool(name="xtlo_dr", bufs=1, space="DRAM")
        XTlo_dr = XTlo_dr_cm.__enter__().tile([7, TS * 128], BF16)

        # ---------------- phase A: build XT ----------------
        with tc.tile_pool(name="pa", bufs=1) as pa, \
             tc.tile_pool(name="paps", bufs=2, space="PSUM") as pa_ps, \
             tc.tile_pool(name="pasb", bufs=3) as pa_sb:
            Xsb = pa.tile([T, BC * D], BF16)
            nc.sync.dma_start(out=Xsb[:].rearrange("p (b d) -> p b d", d=D),
                              in_=poses[:, :, :].rearrange("b t d -> t b d"))
            XThi3 = XThi[:].rearrange("p (t w) -> p t w", w=128)
            XTlo_dr3 = XTlo_dr[:].rearrange("p (t w) -> p t w", w=128)
            for b in range(BC):
                phT = pa_ps.tile([128, T], BF16, tag="phT")
                nc.tensor.transpose(phT[:], Xsb[:, b * D: b * D + 128], ident[0:T, 0:T])
                plT = pa_ps.tile([7, T], BF16, tag="plT")
                nc.tensor.transpose(plT[:], Xsb[:, b * D + 128: (b + 1) * D], ident[0:T, 0:T])
                nc.vector.tensor_copy(XThi3[:, 0:TS, b:b + 1], phT[:, 0:TS, None])
                plo = pa_sb.tile([7, T], BF16, tag="plo")
                nc.vector.tensor_copy(plo[:], plT[:])
                nc.sync.dma_start(out=XTlo_dr3[:, 0:TS, b:b + 1], in_=plo[:, 0:TS, None])

        # ---------------- encode working pools ----------------
        eb_cm = tc.tile_pool(name="eb", bufs=1)     # big fp32 elementwise tiles
        eb = eb_cm.__enter__()
        es_cm = tc.tile_pool(name="es", bufs=2)     # small tiles
        es = es_cm.__enter__()
        ep_cm = tc.tile_pool(name="ep", bufs=1, space="PSUM")
        ep = ep_cm.__enter__()
        ep2_cm = tc.tile_pool(name="ep2", bufs=2, space="PSUM")
        ep2 = ep2_cm.__enter__()
        ed_cm = tc.tile_pool(name="ed", bufs=3, space="DRAM")
        ed = ed_cm.__enter__()
        ea_cm = tc.tile_pool(name="ea", bufs=2)
        ea = ea_cm.__enter__()

        def l0_full(sb, bigsb, ps, ps2, ih_pairs):
            """Full-gate L0 GRU cell (batch 128, natural r|z|n order).
            Updates h0f in place and rebuilds h0T."""
            pR = ps.tile([BC, 1024], F32, tag="pA")
            for rng in range(2):
                sl = slice(rng * 512, (rng + 1) * 512)
                for k in range(8):
                    nc.tensor.matmul(pR[:, sl], h0T[:, k * 128:(k + 1) * 128], whh0_k[k][:, sl],
                                     start=(k == 0), stop=False)
                for lhsT, rhs in ih_pairs:
                    nc.tensor.matmul(pR[:, sl], lhsT, rhs[:, sl], start=False, stop=False)
                nc.tensor.matmul(pR[:, sl], ones_col[:, 0:BC], b0rz_sb[:, sl], start=False, stop=True)
            r_sb = bigsb.tile([BC, H], F32, tag="r_sb")
            nc.scalar.activation(r_sb[:], pR[:], AF.Sigmoid)
            pB = ps.tile([BC, 1024], F32, tag="pB")
            pC = ps.tile([BC, 1024], F32, tag="pC")
            for rng in range(2):
                sl = slice(rng * 512, (rng + 1) * 512)
                slw = slice(2048 + rng * 512, 2048 + (rng + 1) * 512)
                for k in range(8):
                    nc.tensor.matmul(pB[:, sl], h0T[:, k * 128:(k + 1) * 128], whh0_k[k][:, slw],
                                     start=(k == 0), stop=False)
                nc.tensor.matmul(pB[:, sl], ones_col[:, 0:BC], b0nh_sb[:, sl], start=False, stop=True)
                for j, (lhsT, rhs) in enumerate(ih_pairs):
                    nc.tensor.matmul(pC[:, sl], lhsT, rhs[:, slw], start=(j == 0), stop=False)
                nc.tensor.matmul(pC[:, sl], ones_col[:, 0:BC], b0ni_sb[:, sl], start=False, stop=True)
            t1f = bigsb.tile([BC, H], F32, tag="t1f")
            nc.vector.tensor_tensor(out=t1f[:], in0=pB[:], in1=r_sb[:], op=ALU.mult)
            nc.vector.tensor_tensor(out=t1f[:], in0=t1f[:], in1=pC[:], op=ALU.add)
            nnf = bigsb.tile([BC, H], F32, tag="nnf")
            nc.scalar.activation(nnf[:], t1f[:], AF.Tanh)
            pZ = ps.tile([BC, 1024], F32, tag="pA")
            for rng in range(2):
                sl = slice(rng * 512, (rng + 1) * 512)
                slz = slice(1024 + rng * 512, 1024 + (rng + 1) * 512)
                for k in range(8):
                    nc.tensor.matmul(pZ[:, sl], h0T[:, k * 128:(k + 1) * 128], whh0_k[k][:, slz],
                                     start=(k == 0), stop=False)
                for lhsT, rhs in ih_pairs:
                    nc.tensor.matmul(pZ[:, sl], lhsT, rhs[:, slz], start=False, stop=False)
                nc.tensor.matmul(pZ[:, sl], ones_col[:, 0:BC], b0rz_sb[:, 1024 + rng * 512:1024 + (rng + 1) * 512],
                                 start=False, stop=True)
            z_sb = bigsb.tile([BC, H], F32, tag="z_sb")
            nc.scalar.activation(z_sb[:], pZ[:], AF.Sigmoid)
            nc.vector.tensor_tensor(out=t1f[:], in0=h0f[:], in1=nnf[:], op=ALU.subtract)
            nc.vector.tensor_tensor(out=t1f[:], in0=t1f[:], in1=z_sb[:], op=ALU.mult)
            nc.vector.tensor_tensor(out=h0f[:], in0=nnf[:], in1=t1f[:], op=ALU.add)
            hb0 = bigsb.tile([BC, H], BF16, tag="hb0")
            nc.vector.tensor_copy(hb0[:], h0f[:])
            for k in range(8):
                pT = ps2.tile([128, 128], BF16, tag="Tr")
                nc.tensor.transpose(pT[:], hb0[:, k * 128:(k + 1) * 128], ident[:])
                nc.vector.tensor_copy(h0T[:, k * 128:(k + 1) * 128], pT[:])

        def l1_cell(sb, ps, ps2, h1T_cur, agin):
            """Gate-sharded L1 cell; ih from h0T, whh from gathered h1T.
            Stages transposed bf16 h1 slice into agin [128, 256]."""
            psA = ps.tile([BC, 512], F32, tag="pA")
            psBC = ps.tile([BC, 512], F32, tag="pB")
            first = True
            for k in range(8):
                nc.tensor.matmul(psA[:], h1T_cur[:, k * 128:(k + 1) * 128], whh1_k[k][:, 0:512],
                                 start=first, stop=False)
                first = False
            for k in range(8):
                nc.tensor.matmul(psA[:], h0T[:, k * 128:(k + 1) * 128], wih1_k[k][:, 0:512],
                                 start=False, stop=False)
            nc.tensor.matmul(psA[:], ones_col[:, 0:BC], brz1_sb[:], start=False, stop=True)
            for k in range(8):
                nc.tensor.matmul(psBC[:, 0:256], h1T_cur[:, k * 128:(k + 1) * 128], whh1_k[k][:, 512:768],
                                 start=(k == 0), stop=False)
            nc.tensor.matmul(psBC[:, 0:256], ones_col[:, 0:BC], bnh1_sb[:], start=False, stop=True)
            for k in range(8):
                nc.tensor.matmul(psBC[:, 256:512], h0T[:, k * 128:(k + 1) * 128], wih1_k[k][:, 512:768],
                                 start=(k == 0), stop=False)
            nc.tensor.matmul(psBC[:, 256:512], ones_col[:, 0:BC], bni1_sb[:], start=False, stop=True)
            rz = sb.tile([BC, 512], F32, tag="rz")
            nc.scalar.activation(rz[:], psA[:], AF.Sigmoid)
            t1 = sb.tile([BC, 256], F32, tag="t1")
            nc.vector.tensor_tensor(out=t1[:], in0=psBC[:, 0:256], in1=rz[:, 0:256], op=ALU.mult)
            nc.vector.tensor_tensor(out=t1[:], in0=t1[:], in1=psBC[:, 256:512], op=ALU.add)
            nn_ = sb.tile([BC, 256], F32, tag="nn")
            nc.scalar.activation(nn_[:], t1[:], AF.Tanh)
            nc.vector.tensor_tensor(out=t1[:], in0=h1[:], in1=nn_[:], op=ALU.subtract)
            nc.vector.tensor_tensor(out=t1[:], in0=t1[:], in1=rz[:, 256:512], op=ALU.mult)
            nc.vector.tensor_tensor(out=h1[:], in0=nn_[:], in1=t1[:], op=ALU.add)
            hb = sb.tile([BC, 256], BF16, tag="hb")
            nc.vector.tensor_copy(hb[:], h1[:])
            pT = ps2.tile([128, 256], BF16, tag="Tr")
            nc.tensor.transpose(pT[:, 0:128], hb[:, 0:128], ident[:])
            nc.tensor.transpose(pT[:, 128:256], hb[:, 128:256], ident[:])
            nc.vector.tensor_copy(agin[:], pT[:])

        def ag_h1(ag_pool, dr_pool, agin):
            """AllGather the [128,256] bf16 h1 slice -> fresh h1T [128, H]."""
            ag_i = dr_pool.tile([128, 256], BF16, tag="agi")
            ag_o = dr_pool.tile([512, 256], BF16, tag="ago")
            nc.sync.dma_start(out=ag_i[:], in_=agin[:])
            nc.gpsimd.collective_compute("AllGather", ALU.bypass, ins=[ag_i.opt()],
                                         outs=[ag_o.opt()], replica_groups=GROUPS)
            h1T_new = ag_pool.tile([128, H], BF16, tag="hT1")
            nc.sync.dma_start(out=h1T_new[:].rearrange("p (r tl b) -> p r tl b", r=4, tl=2),
                              in_=ag_o[:].rearrange("(r p) (tl b) -> p r tl b", r=4, tl=2))
            return h1T_new

        def load_xlo(t):
            t = t % T
            xlo = es.tile([7, 128], BF16, tag="xlo")
            nc.sync.dma_start(out=xlo[:], in_=XTlo_dr[:, t * 128:(t + 1) * 128])
            return xlo

        def l0_ih(t, xlo_tile):
            t = t % T
            return [(XThi[:, t * 128:(t + 1) * 128], wih0hi[:]), (xlo_tile[:], wih0lo[:])]

        # ---------------- phase C: encode ----------------
        h1T = ea.tile([128, H], BF16, tag="hT1")
        nc.vector.memset(h1T[:], 0.0)

        xlo = load_xlo(0)
        l0_full(es, eb, ep, ep2, l0_ih(0, xlo))
        for t in range(t_steps):
            agin = ea.tile([128, 256], BF16, tag="agin")
            l1_cell(es, ep, ep2, h1T, agin)
            h1T = ag_h1(ea, ed, agin)
            if t + 1 < t_steps:
                xlo = load_xlo(t + 1)
                l0_full(es, eb, ep, ep2, l0_ih(t + 1, xlo))

        # hold final h1T across the encode-pool teardown
        nc.vector.tensor_copy(h1T_hold[:], h1T[:])

        ea_cm.__exit__(None, None, None)
        ed_cm.__exit__(None, None, None)
        ep2_cm.__exit__(None, None, None)
        ep_cm.__exit__(None, None, None)
        es_cm.__exit__(None, None, None)
        eb_cm.__exit__(None, None, None)
        XTlo_dr_cm.__exit__(None, None, None)
        XP_cm.__exit__(None, None, None)

        # ---------------- phase D: rollout ----------------
        rp_cm = tc.tile_pool(name="rp", bufs=1)
        rp = rp_cm.__enter__()
        pre_k = [rp.tile([128, H], BF16, tag=f"pre_{k}", name=f"pre_{k}") for k in range(8)]
        for k in range(8):
            nc.sync.dma_start(out=pre_k[k][:], in_=pre_wT[k * 128:(k + 1) * 128, :])
        pre_b_sb = load(pre_b, pool=rp)
        spl1b_sb = load(spl1b, pool=rp)
        spl2_sb = load(spl2, pool=rp)
        spl2b_sb = load(spl2b, pool=rp)
        p6d = rp.tile([BC, J * 6], F32, tag="p6d")
        nc.sync.dma_start(out=p6d[:], in_=prev6d0[:])
        xTh = rp.tile([128, BC], BF16, tag="xTh")
        xTl = rp.tile([7, BC], BF16, tag="xTl")
        nc.sync.dma_start(out=xTh[:], in_=x0T[0:128, :])
        nc.sync.dma_start(out=xTl[:], in_=x0T[128:D, :])

        rb_cm = tc.tile_pool(name="rb", bufs=1)
        rbp = rb_cm.__enter__()
        rss_cm = tc.tile_pool(name="rss", bufs=2)
        rss = rss_cm.__enter__()
        rps_cm = tc.tile_pool(name="rps", bufs=1, space="PSUM")
        rps = rps_cm.__enter__()
        rps2_cm = tc.tile_pool(name="rps2", bufs=2, space="PSUM")
        rps2 = rps2_cm.__enter__()
        rdr_cm = tc.tile_pool(name="rdr", bufs=3, space="DRAM")
        rdr = rdr_cm.__enter__()
        rag_cm = tc.tile_pool(name="rag", bufs=2)
        rag = rag_cm.__enter__()

        h1T = rag.tile([128, H], BF16, tag="hT1")
        nc.vector.tensor_copy(h1T[:], h1T_hold[:])

        for t in range(pred_steps):
            l0_full(rss, rbp, rps, rps2, [(xTh[:], wih0hi[:]), (xTl[:], wih0lo[:])])
            agin = rag.tile([128, 256], BF16, tag="agin")
            l1_cell(rss, rps, rps2, h1T, agin)
            h1T = ag_h1(rag, rdr, agin)
            # pre + ctx + relu -> hidden [BC, H] (replicated)
            hid = rbp.tile([BC, H], F32, tag="hid")
            for s in range(2):
                sl = slice(s * 512, (s + 1) * 512)
                pP = rps.tile([BC, 512], F32, tag="pA")
                for k in range(8):
                    nc.tensor.matmul(pP[:], h1T[:, k * 128:(k + 1) * 128], pre_k[k][:, sl],
                                     start=(k == 0), stop=False)
                nc.tensor.matmul(pP[:], ones_col[:, 0:BC], pre_b_sb[:, sl], start=False, stop=True)
                nc.scalar.activation(hid[:, sl], pP[:], AF.Relu)
            nc.vector.tensor_tensor(out=hid[:], in0=hid[:], in1=ctx_sb[:], op=ALU.add)
            hidb = rbp.tile([BC, H], BF16, tag="hidb")
            nc.vector.tensor_copy(hidb[:], hid[:])
            hidT = rbp.tile([128, H], BF16, tag="hidT")
            for k in range(8):
                pT = rps2.tile([128, 128], BF16, tag="Tr")
                nc.tensor.transpose(pT[:], hidb[:, k * 128:(k + 1) * 128], ident[:])
                nc.vector.tensor_copy(hidT[:, k * 128:(k + 1) * 128], pT[:])
            # spline layer 1 (weights streamed, two 960-col chunks): z1 [BC, J*128]
            z1 = rbp.tile([BC, J * 128], BF16, tag="z1")
            for ch in range(2):
                d0 = ch * 960
                pZ = rps.tile([BC, 1024], F32, tag="pA")
                regs = ((0, 512), (512, 448))
                for k in range(8):
                    sw = rss.tile([128, 960], BF16, tag="spl1s")
                    nc.sync.dma_start(out=sw[:], in_=spl1T[k * 128:(k + 1) * 128, d0:d0 + 960])
                    for c0, w in regs:
                        nc.tensor.matmul(pZ[:, c0:c0 + w], hidT[:, k * 128:(k + 1) * 128],
                                         sw[:, c0:c0 + w], start=(k == 0), stop=False)
                for c0, w in regs:
                    nc.tensor.matmul(pZ[:, c0:c0 + w], ones_col[:, 0:BC],
                                     spl1b_sb[:, d0 + c0:d0 + c0 + w], start=False, stop=True)
                nc.scalar.activation(z1[:, d0:d0 + 960], pZ[:, 0:960], AF.Relu)
            # spline layer 2 per joint
            pD = rps.tile([BC, J * 6], F32, tag="pC")
            for j in range(J):
                pT = rps2.tile([128, 128], BF16, tag="Tr")
                nc.tensor.transpose(pT[:], z1[:, j * 128:(j + 1) * 128], ident[:])
                z1T = rss.tile([128, BC], BF16, tag="z1T")
                nc.vector.tensor_copy(z1T[:], pT[:])
                nc.tensor.matmul(pD[:, j * 6:(j + 1) * 6], z1T[:], spl2_sb[:, j * 6:(j + 1) * 6],
                                 start=True, stop=False)
                nc.tensor.matmul(pD[:, j * 6:(j + 1) * 6], ones_col[:, 0:BC],
                                 spl2b_sb[:, j * 6:(j + 1) * 6], start=False, stop=True)
            nc.vector.tensor_tensor(out=p6d[:], in0=p6d[:], in1=pD[:], op=ALU.add)
            nc.sync.dma_start(out=out6d[t % PRED, :, :], in_=p6d[:])
            # rot6d -> R -> x_new -> transposed x for next step
            v6 = p6d[:].rearrange("p (j r w) -> p j r w", r=3, w=2)
            a1, a2 = v6[:, :, :, 0], v6[:, :, :, 1]
            t12 = rss.tile([BC, J * 3], F32, tag="t12")
            t12v = t12[:].rearrange("p (j c) -> p j c", c=3)
            n4 = rss.tile([BC, J], F32, tag="n4")
            r4_ = rss.tile([BC, J], F32, tag="r4_")
            B1 = rss.tile([BC, J * 3], F32, tag="B1")
            B1v = B1[:].rearrange("p (j c) -> p j c", c=3)
            B2 = rss.tile([BC, J * 3], F32, tag="B2")
            B2v = B2[:].rearrange("p (j c) -> p j c", c=3)
            B3 = rss.tile([BC, J * 3], F32, tag="B3")
            B3v = B3[:].rearrange("p (j c) -> p j c", c=3)

            def normize(src, dst):
                nc.vector.tensor_tensor(out=t12v, in0=src, in1=src, op=ALU.mult)
                nc.vector.reduce_sum(n4[:, :, None], t12v, axis=mybir.AxisListType.X)
                nc.scalar.activation(n4[:], n4[:], AF.Sqrt)
                nc.vector.tensor_scalar_max(n4[:], n4[:], 1e-12)
                nc.vector.reciprocal(r4_[:], n4[:])
                nc.vector.tensor_tensor(out=dst, in0=src,
                                        in1=r4_[:, :, None].broadcast_to([BC, J, 3]), op=ALU.mult)

            normize(a1, B1v)
            nc.vector.tensor_tensor(out=t12v, in0=B1v, in1=a2, op=ALU.mult)
            nc.vector.reduce_sum(n4[:, :, None], t12v, axis=mybir.AxisListType.X)
            nc.vector.tensor_tensor(out=t12v, in0=B1v,
                                    in1=n4[:, :, None].broadcast_to([BC, J, 3]), op=ALU.mult)
            a2o = rss.tile([BC, J * 3], F32, tag="a2o")
            a2ov = a2o[:].rearrange("p (j c) -> p j c", c=3)
            nc.vector.tensor_tensor(out=a2ov, in0=a2, in1=t12v, op=ALU.subtract)
            normize(a2ov, B2v)
            for c in range(3):
                u, v = (c + 1) % 3, (c + 2) % 3
                m1 = rss.tile([BC, J], F32, tag="m1")
                nc.vector.tensor_tensor(out=m1[:, :, None], in0=B1v[:, :, u:u + 1],
                                        in1=B2v[:, :, v:v + 1], op=ALU.mult)
                m2 = rss.tile([BC, J], F32, tag="m2")
                nc.vector.tensor_tensor(out=m2[:, :, None], in0=B1v[:, :, v:v + 1],
                                        in1=B2v[:, :, u:u + 1], op=ALU.mult)
                nc.vector.tensor_tensor(out=B3v[:, :, c:c + 1], in0=m1[:, :, None],
                                        in1=m2[:, :, None], op=ALU.subtract)
            xn = rss.tile([BC, J * 9], BF16, tag="xn")
            xnv = xn[:].rearrange("p (j r c) -> p j r c", r=3, c=3)
            nc.vector.tensor_copy(xnv[:, :, :, 0:1], B1v[:, :, :, None])
            nc.vector.tensor_copy(xnv[:, :, :, 1:2], B2v[:, :, :, None])
            nc.vector.tensor_copy(xnv[:, :, :, 2:3], B3v[:, :, :, None])
            pT = rps2.tile([128, 128], BF16, tag="Tr")
            nc.tensor.transpose(pT[:], xn[:, 0:128], ident[:])
            nc.vector.tensor_copy(xTh[:], pT[:])
            pT2 = rps2.tile([7, 128], BF16, tag="Tr")
            nc.tensor.transpose(pT2[:], xn[:, 128:D], ident[:])
            nc.vector.tensor_copy(xTl[:], pT2[:])

        rag_cm.__exit__(None, None, None)
        rdr_cm.__exit__(None, None, None)
        rps2_cm.__exit__(None, None, None)
        rps_cm.__exit__(None, None, None)
        rss_cm.__exit__(None, None, None)
        rb_cm.__exit__(None, None, None)
        rp_cm.__exit__(None, None, None)
        wq_cm.__exit__(None, None, None)
        wp_cm.__exit__(None, None, None)
        rs_cm.__exit__(None, None, None)
    nc.compile()
    return nc


# ---------------- host side ----------------
_cached = {}


class _SpmdRunner:
    def __init__(self, nc, n_cores):
        import jax
        from jax.sharding import Mesh, PartitionSpec
        from jax.experimental.shard_map import shard_map
        from concourse import bass2jax
        from concourse.bass2jax import _bass_exec_p, partition_id_tensor
        bass2jax.install_neuronx_cc_hook()
        self.jax = jax
        self.PartitionSpec = PartitionSpec
        self.n_cores = n_cores
        in_names, out_names, out_avals, zero_outs = [], [], [], []
        pname = nc.partition_id_tensor.name if nc.partition_id_tensor else None
        for alloc in nc.m.functions[0].allocations:
            if not isinstance(alloc, mybir.MemoryLocationSet):
                continue
            name = alloc.memorylocations[0].name
            if alloc.kind == "ExternalInput":
                if name != pname:
                    in_names.append(name)
            elif alloc.kind == "ExternalOutput":
                out_names.append(name)
                shape = tuple(alloc.tensor_shape)
                dtype = mybir.dt.np(alloc.dtype)
                out_avals.append(jax.core.ShapedArray(shape, dtype))
                zero_outs.append(np.zeros(shape, dtype))
        self.in_names, self.out_names = in_names, out_names
        self.out_avals, self.zero_outs = out_avals, zero_outs
        n_params, n_outs = len(in_names), len(out_names)
        all_in = in_names + out_names + ([pname] if pname else [])

        def _body(*args):
            operands = list(args)
            if pname is not None:
                operands.append(partition_id_tensor())
            return tuple(_bass_exec_p.bind(
                *operands, out_avals=tuple(out_avals), in_names=tuple(all_in),
                out_names=tuple(out_names), lowering_input_output_aliases=(),
                sim_require_finite=True, sim_require_nnan=True, nc=nc))

        devices = jax.devices()[:n_cores]
        self.mesh = Mesh(np.asarray(devices), ("core",))
        specs = (PartitionSpec("core"),) * (n_params + n_outs)
        self.fn = jax.jit(shard_map(_body, mesh=self.mesh, in_specs=specs,
                                    out_specs=(PartitionSpec("core"),) * n_outs,
                                    check_rep=False), keep_unused=True)

    def put(self, in_maps):
        import jax
        from jax.sharding import NamedSharding
        sh = NamedSharding(self.mesh, self.PartitionSpec("core"))
        args = []
        for name in self.in_names:
            arr = np.concatenate([np.asarray(m[name]) for m in in_maps], axis=0)
            args.append(jax.device_put(arr, sh))
        for z in self.zero_outs:
            args.append(jax.device_put(np.concatenate([z] * self.n_cores, axis=0), sh))
        return args

    def run(self, args):
        import jax
        outs = self.fn(*args)
        jax.block_until_ready(outs)
        return outs

    def results(self, outs):
        res = []
        for c in range(self.n_cores):
            d = {}
            for i, name in enumerate(self.out_names):
                d[name] = np.asarray(outs[i]).reshape(self.n_cores, *self.out_avals[i].shape)[c]
            res.append(d)
        return res


def get_runner(t_steps=T_STEPS, pred_steps=PRED_STEPS):
    key = (t_steps, pred_steps)
    if key not in _cached:
        nc = build_module(t_steps, pred_steps)
        _cached[key] = _SpmdRunner(nc, 8)
    return _cached[key]


def make_in_maps(inputs):
    poses = np.asarray(inputs["poses"], np.float32)
    freq_w, freq_b = inputs["freq_w"], inputs["freq_b"]
    attn_in_w, attn_in_b = inputs["attn_in_w"], inputs["attn_in_b"]
    attn_out_w, attn_out_b = inputs["attn_out_w"], inputs["attn_out_b"]
    Wf = (freq_w.T.astype(np.float64) @ attn_in_w.T.astype(np.float64)).astype(np.float32)
    bfull = (freq_b.astype(np.float64) @ attn_in_w.T.astype(np.float64)).astype(np.float32) + attn_in_b
    k_ = np.arange(FQ)[None, :]
    t_ = np.arange(T)[:, None]
    ct = np.cos(2 * np.pi * k_ * t_ / T).astype(np.float32)  # [T, FQ]

    x0 = poses[:, T - 1, :]
    R0 = x0.reshape(B, J, 3, 3)
    prev6d0 = np.concatenate([R0[..., 0], R0[..., 1]], axis=-1).reshape(B, J * 6)

    def hsl(w, l):  # w [.., 3072] -> L1 gate slice cols for lane l
        r = w[..., l * 256:(l + 1) * 256]
        z = w[..., 1024 + l * 256:1024 + (l + 1) * 256]
        n = w[..., 2048 + l * 256:2048 + (l + 1) * 256]
        return np.concatenate([r, z, n], axis=-1)

    def bsl(b1, b2, l):
        s = b1 + b2
        return (np.concatenate([s[l * 256:(l + 1) * 256],
                                s[1024 + l * 256:1024 + (l + 1) * 256]])[None],
                b1[2048 + l * 256:2048 + (l + 1) * 256][None],
                b2[2048 + l * 256:2048 + (l + 1) * 256][None])

    b0s = inputs["gru_bih0"] + inputs["gru_bhh0"]
    spl1T_f = _bf(np.concatenate([inputs["spl_w1"][j].T for j in range(J)], axis=1))
    spl1b_f = _bf(np.concatenate([inputs["spl_b1"][j] for j in range(J)])[None])
    spl2_f = _bf(np.concatenate([inputs["spl_w2"][j].T for j in range(J)], axis=1))
    spl2b_f = _bf(np.concatenate([inputs["spl_b2"][j] for j in range(J)])[None])

    in_maps = []
    for c in range(8):
        g, l = c // 4, c % 4
        bs = slice(g * BC, (g + 1) * BC)
        wfh = np.concatenate([Wf[:, l * 256:(l + 1) * 256],
                              Wf[:, 1024 + l * 256:1024 + (l + 1) * 256],
                              Wf[:, 2048 + l * 256:2048 + (l + 1) * 256]], axis=1)
        bfh = np.concatenate([bfull[l * 256:(l + 1) * 256],
                              bfull[1024 + l * 256:1024 + (l + 1) * 256],
                              bfull[2048 + l * 256:2048 + (l + 1) * 256]])[None]
        brz1_, bni1_, bnh1_ = bsl(inputs["gru_bih1"], inputs["gru_bhh1"], l)
        m = {
            "poses_b": _bf(poses[bs, :T, :]),
            "poses32": poses[bs, :T, :].astype(np.float32),
            "ct32": ct,
            "wf_qk32": wfh.astype(np.float32),
            "bf_qk32": bfh.astype(np.float32),
            "wout_h": attn_out_w[:, l * 256:(l + 1) * 256].T.astype(np.float32),
            "aob4": (attn_out_b / 4.0)[None].astype(np.float32),
            "wih0f": _bf(inputs["gru_wih0"].T),
            "whh0f": _bf(inputs["gru_whh0"].T),
            "b0rz": _bf(b0s[0:2 * H][None]),
            "b0ni": _bf(inputs["gru_bih0"][2 * H:3 * H][None]),
            "b0nh": _bf(inputs["gru_bhh0"][2 * H:3 * H][None]),
            "wih1T_s": _bf(hsl(inputs["gru_wih1"].T, l)),
            "whh1T_s": _bf(hsl(inputs["gru_whh1"].T, l)),
            "brz1": _bf(brz1_), "bni1": _bf(bni1_), "bnh1": _bf(bnh1_),
            "pre_wT": _bf(inputs["pre_w"].T), "pre_b_row": _bf(inputs["pre_b"][None]),
            "spl1T_f": spl1T_f, "spl1b_f": spl1b_f,
            "spl2_f": spl2_f, "spl2b_f": spl2b_f,
            "x0T": _bf(x0[bs].T),
            "prev6d0_s": prev6d0[bs].astype(np.float32),
        }
        in_maps.append(m)
    return in_maps


def assemble_output(res, pred_steps=PRED_STEPS):
    pred6d = np.zeros((B, pred_steps, J * 6), np.float32)
    for g in range(2):
        o = res[g * 4]["out6d"][:pred_steps]          # [pred, BC, J*6] from core 0 / 4
        pred6d[g * BC:(g + 1) * BC] = o.transpose(1, 0, 2)
    return pred6d


def kernel(**inputs):
    runner = get_runner()
    in_maps = make_in_maps(inputs)
    args = runner.put(in_maps)
    res = runner.results(runner.run(args))
    return assemble_output(res)


# revision 4
# speedup vs baseline: 1.1542x; 1.1542x over previous
"""Trainium2 Bass kernel v3 for nn_BaseModel_32255204393001.

Sharding (8 cores): batch 256 -> 2 groups of 128 (cores 0-3 / 4-7).
Asymmetric GRU split: L0 computed FULL (replicated per core, batch 128) --
its ~14us of PE work fills the AllGather window and keeps the PE warm; L1
gate-sharded 4-way; ONE AllGather per step carrying only h1 slices
([128,256] bf16, out 256KB). 120 encode + 24 rollout AGs total.
Rollout: L0-full + L1-sharded + AG(h1) + replicated spline (no x gather).
Attention in bf16 (fp32 softmax internals). bf16 matmuls, fp32 state/PSUM.
"""

import numpy as np
import ml_dtypes

import concourse.bacc as bacc
import concourse.mybir as mybir
import concourse.tile as tile
from concourse.masks import make_identity

F32 = mybir.dt.float32
BF16 = mybir.dt.bfloat16
AF = mybir.ActivationFunctionType
ALU = mybir.AluOpType

B, T, PRED, J, H, D = 256, 120, 24, 15, 1024, 135
HEADS = 4
FQ = T // 2 + 1          # 61 freq bins
BC = 128                 # batch per group
GSL = 768                # L1 gate slice per lane (r|z|n 256 each)
CA = 16                  # attention batch sub-chunk (freq stage)
CB = 32                  # attention batch chunk
GROUPS = [[0, 1, 2, 3], [4, 5, 6, 7]]

T_STEPS = T
PRED_STEPS = PRED


def _bf(x):
    return np.asarray(x, dtype=ml_dtypes.bfloat16)


def build_module(t_steps=T_STEPS, pred_steps=PRED_STEPS):
    nc = bacc.Bacc("TRN2", target_bir_lowering=False, debug=False, num_devices=8)

    def din(name, shape, dt=BF16):
        return nc.dram_tensor(name, shape, dt, kind="ExternalInput")

    # ---- attention inputs (fp32, per-lane head slice) ----
    poses = din("poses_b", [BC, T, D])
    wf_qk32 = din("wf_qk32", [D, 768], F32)
    poses32 = din("poses32", [BC, T, D], F32)
    ct32 = din("ct32", [T, FQ], F32)
    bf_qk32 = din("bf_qk32", [1, 768], F32)
    wout_h = din("wout_h", [256, H], F32)
    aob4 = din("aob4", [1, H], F32)
    # ---- L0 full weights (natural gate order r|z|n) ----
    wih0f = din("wih0f", [D, 3 * H])
    whh0f = din("whh0f", [H, 3 * H])
    b0rz = din("b0rz", [1, 2 * H]); b0ni = din("b0ni", [1, H]); b0nh = din("b0nh", [1, H])
    # ---- L1 sharded weights (per-lane gate slices r|z|n 256 each) ----
    wih1T = din("wih1T_s", [H, GSL])
    whh1T = din("whh1T_s", [H, GSL])
    brz1 = din("brz1", [1, 512]); bni1 = din("bni1", [1, 256]); bnh1 = din("bnh1", [1, 256])
    # ---- rollout spline weights (replicated) ----
    pre_wT = din("pre_wT", [H, H])
    pre_b = din("pre_b_row", [1, H])
    spl1T = din("spl1T_f", [H, J * 128])
    spl1b = din("spl1b_f", [1, J * 128])
    spl2 = din("spl2_f", [128, J * 6])
    spl2b = din("spl2b_f", [1, J * 6])
    # ---- rollout init (group batch 128) ----
    x0T = din("x0T", [D, BC])
    prev6d0 = din("prev6d0_s", [BC, J * 6], F32)

    out6d = nc.dram_tensor("out6d", [PRED, BC, J * 6], F32, kind="ExternalOutput")

    TS = min(t_steps, T)

    with tile.TileContext(nc) as tc:
        # ---------------- rs: whole-kernel constants ----------------
        rs_cm = tc.tile_pool(name="rs", bufs=1)
        rs = rs_cm.__enter__()
        ident = rs.tile([128, 128], BF16)
        make_identity(nc, ident)
        ones_col = rs.tile([1, 128], BF16)
        nc.vector.memset(ones_col[:], 1.0)
        ctx_sb = rs.tile([BC, H], F32, tag="ctx_sb")

        # ---------------- wp: weights + persistent state ----------------
        wp_cm = tc.tile_pool(name="wp", bufs=1)
        wp = wp_cm.__enter__()
        ones32 = wp.tile([1, 512], F32, tag="ones32")
        nc.vector.memset(ones32[:], 1.0)
        ones61_32 = wp.tile([FQ, 1], F32, tag="ones61_32")
        nc.vector.memset(ones61_32[:], 1.0 / FQ)

        def load(t_dram, dt=BF16, pool=None, tag=None):
            p = pool or wp
            tl = p.tile(list(t_dram.shape), dt, tag=tag or t_dram.name)
            nc.sync.dma_start(out=tl[:], in_=t_dram[:])
            return tl

        wih0hi = wp.tile([128, 3 * H], BF16, tag="wih0hi")
        nc.sync.dma_start(out=wih0hi[:], in_=wih0f[0:128, :])
        wih0lo = wp.tile([7, 3 * H], BF16, tag="wih0lo")
        nc.sync.dma_start(out=wih0lo[:], in_=wih0f[128:D, :])
        b0rz_sb = load(b0rz); b0ni_sb = load(b0ni); b0nh_sb = load(b0nh)
        brz1_sb = load(brz1); bni1_sb = load(bni1); bnh1_sb = load(bnh1)

        # state: h0 full (replicated), h1 own lane slice
        h0f = wp.tile([BC, H], F32, tag="h0f"); nc.vector.memset(h0f[:], 0.0)
        h0T = wp.tile([128, H], BF16, tag="h0T"); nc.vector.memset(h0T[:], 0.0)
        h1 = wp.tile([BC, 256], F32, tag="h1"); nc.vector.memset(h1[:], 0.0)
        h1T_hold = wp.tile([128, H], BF16, tag="h1T_hold")

        # ---------------- phase B: attention ----------------
        with tc.tile_pool(name="pb", bufs=1) as pb, \
             tc.tile_pool(name="pb2", bufs=1) as pb2, \
             tc.tile_pool(name="ps1", bufs=1, space="PSUM") as ps1:
            ct32_sb = load(ct32, dt=F32, pool=pb)
            wfq32hi = pb.tile([128, 768], F32, tag="wfq32hi")
            nc.sync.dma_start(out=wfq32hi[:], in_=wf_qk32[0:128, :])
            wfq32lo = pb.tile([7, 768], F32, tag="wfq32lo")
            nc.sync.dma_start(out=wfq32lo[:], in_=wf_qk32[128:D, :])
            bfqk32_sb = load(bf_qk32, dt=F32, pool=pb)
            wout0 = pb.tile([128, H], F32, tag="wout0")
            nc.sync.dma_start(out=wout0[:], in_=wout_h[0:128, :])
            wout1 = pb.tile([128, H], F32, tag="wout1")
            nc.sync.dma_start(out=wout1[:], in_=wout_h[128:256, :])
            aob_sb = load(aob4, dt=F32, pool=pb)

            O_hi = pb.tile([128, BC], F32, tag="O_hi")
            O_lo = pb.tile([128, BC], F32, tag="O_lo")
            for ci in range(BC // CB):
                fq32hi = pb2.tile([128, FQ * CB], F32, tag="fq32hi")
                fq32lo = pb2.tile([7, FQ * CB], F32, tag="fq32lo")
                XC32 = pb2.tile([T, CB * D], F32, tag="XC32")
                nc.sync.dma_start(out=XC32[:].rearrange("p (b d) -> p b d", d=D),
                                  in_=poses32[ci * CB:(ci + 1) * CB, :, :].rearrange("b t d -> t b d"))
                for bi in range(CB):
                    pdd = ps1.tile([128, 128], F32, tag="pdd")
                    nc.tensor.matmul(pdd[:, 0:FQ], XC32[:, bi * D: bi * D + 128], ct32_sb[:], start=True, stop=True)
                    nc.tensor.matmul(pdd[0:7, 64:64 + FQ], XC32[:, bi * D + 128: (bi + 1) * D], ct32_sb[:], start=True, stop=True)
                    nc.vector.tensor_copy(fq32hi[:, bi * FQ:(bi + 1) * FQ], pdd[:, 0:FQ])
                    nc.vector.tensor_copy(fq32lo[:, bi * FQ:(bi + 1) * FQ], pdd[0:7, 64:64 + FQ])
                qk = [pb2.tile([128, FQ * CB], F32, tag=f"qk{m}", name=f"qk{m}") for m in range(4)]
                NCB = FQ * CB  # 1952
                for m in range(4):
                    for s in range(4):
                        c0 = s * 488
                        c1 = min(c0 + 488, NCB)
                        pq = ps1.tile([128, 488], F32, tag="big")
                        nc.tensor.matmul(pq[:, 0:c1 - c0], wfq32hi[:, m * 128:(m + 1) * 128], fq32hi[:, c0:c1], start=True, stop=False)
                        nc.tensor.matmul(pq[:, 0:c1 - c0], wfq32lo[:, m * 128:(m + 1) * 128], fq32lo[:, c0:c1], start=False, stop=False)
                        nc.tensor.matmul(pq[:, 0:c1 - c0], bfqk32_sb[:, m * 128:(m + 1) * 128], ones32[:, 0:c1 - c0], start=False, stop=True)
                        nc.vector.tensor_copy(qk[m][:, c0:c1], pq[:, 0:c1 - c0])
                S_sb = pb2.tile([FQ, FQ * CB], F32, tag="S_sb")
                for bg in range(CB // 8):
                    pS = ps1.tile([FQ, 8 * FQ], F32, tag="S")
                    for bi8 in range(8):
                        bi = bg * 8 + bi8
                        sl = slice(bi * FQ, (bi + 1) * FQ)
                        psl = slice(bi8 * FQ, (bi8 + 1) * FQ)
                        nc.tensor.matmul(pS[:, psl], qk[0][:, sl], qk[2][:, sl], start=True, stop=False)
                        nc.tensor.matmul(pS[:, psl], qk[1][:, sl], qk[3][:, sl], start=False, stop=True)
                    nc.vector.tensor_copy(S_sb[:, bg * 8 * FQ:(bg + 1) * 8 * FQ], pS[:])
                S3 = S_sb[:].rearrange("p (b k) -> p b k", k=FQ)
                mx = pb.tile([FQ, CB], F32, tag="mx")
                nc.vector.reduce_max(mx[:, :, None], S3, axis=mybir.AxisListType.X)
                nc.vector.tensor_tensor(out=S3, in0=S3,
                                        in1=mx[:, :, None].broadcast_to([FQ, CB, FQ]),
                                        op=ALU.subtract)
                nc.scalar.activation(S_sb[:], S_sb[:], AF.Exp, scale=1.0 / 16.0)
                sm = pb.tile([FQ, CB], F32, tag="sm")
                nc.vector.reduce_sum(sm[:, :, None], S3, axis=mybir.AxisListType.X)
                rs = pb.tile([FQ, CB], F32, tag="rs")
                nc.vector.reciprocal(rs[:], sm[:])
                Wt = pb2.tile([FQ, FQ * CB], F32, tag="Wt")
                nc.vector.tensor_tensor(out=Wt[:].rearrange("p (b k) -> p b k", k=FQ),
                                        in0=S3,
                                        in1=rs[:, :, None].broadcast_to([FQ, CB, FQ]), op=ALU.mult)
                pam = ps1.tile([FQ, CB], F32, tag="am")
                for bi in range(CB):
                    nc.tensor.matmul(pam[:, bi:bi + 1], Wt[:, bi * FQ:(bi + 1) * FQ], ones61_32[:],
                                     start=True, stop=True)
                A_sb = pb.tile([FQ, CB], F32, tag="A_sb")
                nc.vector.tensor_copy(A_sb[:], pam[:])
                pOh = ps1.tile([128, CB], F32, tag="pOh")
                pOl = ps1.tile([128, CB], F32, tag="pOl")
                for bi in range(CB):
                    sl = slice(bi * FQ, (bi + 1) * FQ)
                    pv = ps1.tile([FQ, 256], F32, tag="pv")
                    nc.tensor.matmul(pv[:], fq32hi[:, sl], wfq32hi[:, 512:768], start=True, stop=False)
                    nc.tensor.matmul(pv[:], fq32lo[:, sl], wfq32lo[:, 512:768], start=False, stop=False)
                    nc.tensor.matmul(pv[:], ones32[:, 0:FQ], bfqk32_sb[:, 512:768], start=False, stop=True)
                    v_sb = pb2.tile([FQ, 256], F32, tag="v_sb")
                    nc.vector.tensor_copy(v_sb[:], pv[:])
                    nc.tensor.matmul(pOh[:, bi:bi + 1], v_sb[:, 0:128], A_sb[:, bi:bi + 1],
                                     start=True, stop=True)
                    nc.tensor.matmul(pOl[:, bi:bi + 1], v_sb[:, 128:256], A_sb[:, bi:bi + 1],
                                     start=True, stop=True)
                nc.vector.tensor_copy(O_hi[:, ci * CB:(ci + 1) * CB], pOh[:])
                nc.vector.tensor_copy(O_lo[:, ci * CB:(ci + 1) * CB], pOl[:])
            arin = pb.tile([BC, H], F32, tag="arin")
            for s in range(2):
                sl = slice(s * 512, (s + 1) * 512)
                pc = ps1.tile([BC, 512], F32, tag="big")
                nc.tensor.matmul(pc[:], O_hi[:], wout0[:, sl], start=True, stop=False)
                nc.tensor.matmul(pc[:], O_lo[:], wout1[:, sl], start=False, stop=False)
                nc.tensor.matmul(pc[:], ones32[:, 0:128], aob_sb[:, sl], start=False, stop=True)
                nc.vector.tensor_copy(arin[:, sl], pc[:])
            with tc.tile_pool(name="drA", bufs=1, space="DRAM") as drA:
                ar_in = drA.tile([BC, H], F32)
                ar_out = drA.tile([BC, H], F32)
                nc.sync.dma_start(out=ar_in[:], in_=arin[:])
                nc.gpsimd.collective_compute("AllReduce", ALU.add, ins=[ar_in.opt()],
                                             outs=[ar_out.opt()], replica_groups=GROUPS)
                nc.sync.dma_start(out=ctx_sb[:], in_=ar_out[:])

        # ---------------- wq: big GRU weights (after attention frees SBUF) ----
        wq_cm = tc.tile_pool(name="wq", bufs=1)
        wq = wq_cm.__enter__()
        whh0_k = [wq.tile([128, 3 * H], BF16, tag=f"whh0_{k}", name=f"whh0_{k}") for k in range(8)]
        wih1_k = [wq.tile([128, GSL], BF16, tag=f"wih1_{k}", name=f"wih1_{k}") for k in range(8)]
        whh1_k = [wq.tile([128, GSL], BF16, tag=f"whh1_{k}", name=f"whh1_{k}") for k in range(8)]
        for k in range(8):
            nc.sync.dma_start(out=whh0_k[k][:], in_=whh0f[k * 128:(k + 1) * 128, :])
            nc.sync.dma_start(out=wih1_k[k][:], in_=wih1T[k * 128:(k + 1) * 128, :])
            nc.sync.dma_start(out=whh1_k[k][:], in_=whh1T[k * 128:(k + 1) * 128, :])

        # ---------------- encode-only pools ----------------
        XP_cm = tc.tile_pool(name="XP", bufs=1)
        XP = XP_cm.__enter__()
        XThi = XP.tile([128, TS * 128], BF16, tag="XThi")
        XTlo_dr_cm = tc.tile_pool(name="xtlo_dr", bufs=1, space="DRAM")
        XTlo_dr = XTlo_dr_cm.__enter__().tile([7, TS * 128], BF16)

        # ---------------- phase A: build XT ----------------
        with tc.tile_pool(name="pa", bufs=1) as pa, \
             tc.tile_pool(name="paps", bufs=2, space="PSUM") as pa_ps, \
             tc.tile_pool(name="pasb", bufs=3) as pa_sb:
            Xsb = pa.tile([T, BC * D], BF16)
            nc.sync.dma_start(out=Xsb[:].rearrange("p (b d) -> p b d", d=D),
                              in_=poses[:, :, :].rearrange("b t d -> t b d"))
            XThi3 = XThi[:].rearrange("p (t w) -> p t w", w=128)
            XTlo_dr3 = XTlo_dr[:].rearrange("p (t w) -> p t w", w=128)
            for b in range(BC):
                phT = pa_ps.tile([128, T], BF16, tag="phT")
                nc.tensor.transpose(phT[:], Xsb[:, b * D: b * D + 128], ident[0:T, 0:T])
                plT = pa_ps.tile([7, T], BF16, tag="plT")
                nc.tensor.transpose(plT[:], Xsb[:, b * D + 128: (b + 1) * D], ident[0:T, 0:T])
                nc.vector.tensor_copy(XThi3[:, 0:TS, b:b + 1], phT[:, 0:TS, None])
                plo = pa_sb.tile([7, T], BF16, tag="plo")
                nc.vector.tensor_copy(plo[:], plT[:])
                nc.sync.dma_start(out=XTlo_dr3[:, 0:TS, b:b + 1], in_=plo[:, 0:TS, None])

        # ---------------- encode working pools ----------------
        eb_cm = tc.tile_pool(name="eb", bufs=1)     # big fp32 elementwise tiles
        eb = eb_cm.__enter__()
        es_cm = tc.tile_pool(name="es", bufs=2)     # small tiles
        es = es_cm.__enter__()
        ep_cm = tc.tile_pool(name="ep", bufs=1, space="PSUM")
        ep = ep_cm.__enter__()
        ep2_cm = tc.tile_pool(name="ep2", bufs=2, space="PSUM")
        ep2 = ep2_cm.__enter__()
        ed_cm = tc.tile_pool(name="ed", bufs=3, space="DRAM")
        ed = ed_cm.__enter__()
        ea_cm = tc.tile_pool(name="ea", bufs=2)
        ea = ea_cm.__enter__()

        def l0_full(sb, bigsb, ps, ps2, ih_pairs):
            """Full-gate L0 GRU cell (batch 128, natural r|z|n order).
            Updates h0f in place and rebuilds h0T."""
            pR = ps.tile([BC, 1024], F32, tag="pA")
            for rng in range(2):
                sl = slice(rng * 512, (rng + 1) * 512)
                for k in range(8):
                    nc.tensor.matmul(pR[:, sl], h0T[:, k * 128:(k + 1) * 128], whh0_k[k][:, sl],
                                     start=(k == 0), stop=False)
                for lhsT, rhs in ih_pairs:
                    nc.tensor.matmul(pR[:, sl], lhsT, rhs[:, sl], start=False, stop=False)
                nc.tensor.matmul(pR[:, sl], ones_col[:, 0:BC], b0rz_sb[:, sl], start=False, stop=True)
            r_sb = bigsb.tile([BC, H], F32, tag="r_sb")
            nc.scalar.activation(r_sb[:], pR[:], AF.Sigmoid)
            pB = ps.tile([BC, 1024], F32, tag="pB")
            pC = ps.tile([BC, 1024], F32, tag="pC")
            for rng in range(2):
                sl = slice(rng * 512, (rng + 1) * 512)
                slw = slice(2048 + rng * 512, 2048 + (rng + 1) * 512)
                for k in range(8):
                    nc.tensor.matmul(pB[:, sl], h0T[:, k * 128:(k + 1) * 128], whh0_k[k][:, slw],
                                     start=(k == 0), stop=False)
                nc.tensor.matmul(pB[:, sl], ones_col[:, 0:BC], b0nh_sb[:, sl], start=False, stop=True)
                for j, (lhsT, rhs) in enumerate(ih_pairs):
                    nc.tensor.matmul(pC[:, sl], lhsT, rhs[:, slw], start=(j == 0), stop=False)
                nc.tensor.matmul(pC[:, sl], ones_col[:, 0:BC], b0ni_sb[:, sl], start=False, stop=True)
            t1f = bigsb.tile([BC, H], F32, tag="t1f")
            nc.vector.tensor_tensor(out=t1f[:], in0=pB[:], in1=r_sb[:], op=ALU.mult)
            nc.vector.tensor_tensor(out=t1f[:], in0=t1f[:], in1=pC[:], op=ALU.add)
            nnf = bigsb.tile([BC, H], F32, tag="nnf")
            nc.scalar.activation(nnf[:], t1f[:], AF.Tanh)
            pZ = ps.tile([BC, 1024], F32, tag="pA")
            for rng in range(2):
                sl = slice(rng * 512, (rng + 1) * 512)
                slz = slice(1024 + rng * 512, 1024 + (rng + 1) * 512)
                for k in range(8):
                    nc.tensor.matmul(pZ[:, sl], h0T[:, k * 128:(k + 1) * 128], whh0_k[k][:, slz],
                                     start=(k == 0), stop=False)
                for lhsT, rhs in ih_pairs:
                    nc.tensor.matmul(pZ[:, sl], lhsT, rhs[:, slz], start=False, stop=False)
                nc.tensor.matmul(pZ[:, sl], ones_col[:, 0:BC], b0rz_sb[:, 1024 + rng * 512:1024 + (rng + 1) * 512],
                                 start=False, stop=True)
            z_sb = bigsb.tile([BC, H], F32, tag="z_sb")
            nc.scalar.activation(z_sb[:], pZ[:], AF.Sigmoid)
            nc.vector.tensor_tensor(out=t1f[:], in0=h0f[:], in1=nnf[:], op=ALU.subtract)
            nc.vector.tensor_tensor(out=t1f[:], in0=t1f[:], in1=z_sb[:], op=ALU.mult)
            nc.vector.tensor_tensor(out=h0f[:], in0=nnf[:], in1=t1f[:], op=ALU.add)
            hb0 = bigsb.tile([BC, H], BF16, tag="hb0")
            nc.vector.tensor_copy(hb0[:], h0f[:])
            for k in range(8):
                pT = ps2.tile([128, 128], BF16, tag="Tr")
                nc.tensor.transpose(pT[:], hb0[:, k * 128:(k + 1) * 128], ident[:])
                nc.vector.tensor_copy(h0T[:, k * 128:(k + 1) * 128], pT[:])

        def l1_cell(sb, ps, ps2, h1T_cur, agin):
            """Gate-sharded L1 cell; ih from h0T, whh from gathered h1T.
            Stages transposed bf16 h1 slice into agin [128, 256]."""
            psA = ps.tile([BC, 512], F32, tag="pA")
            psBC = ps.tile([BC, 512], F32, tag="pB")
            first = True
            for k in range(8):
                nc.tensor.matmul(psA[:], h1T_cur[:, k * 128:(k + 1) * 128], whh1_k[k][:, 0:512],
                                 start=first, stop=False)
                first = False
            for k in range(8):
                nc.tensor.matmul(psA[:], h0T[:, k * 128:(k + 1) * 128], wih1_k[k][:, 0:512],
                                 start=False, stop=False)
            nc.tensor.matmul(psA[:], ones_col[:, 0:BC], brz1_sb[:], start=False, stop=True)
            for k in range(8):
                nc.tensor.matmul(psBC[:, 0:256], h1T_cur[:, k * 128:(k + 1) * 128], whh1_k[k][:, 512:768],
                                 start=(k == 0), stop=False)
            nc.tensor.matmul(psBC[:, 0:256], ones_col[:, 0:BC], bnh1_sb[:], start=False, stop=True)
            for k in range(8):
                nc.tensor.matmul(psBC[:, 256:512], h0T[:, k * 128:(k + 1) * 128], wih1_k[k][:, 512:768],
                                 start=(k == 0), stop=False)
            nc.tensor.matmul(psBC[:, 256:512], ones_col[:, 0:BC], bni1_sb[:], start=False, stop=True)
            rz = sb.tile([BC, 512], F32, tag="rz")
            nc.scalar.activation(rz[:], psA[:], AF.Sigmoid)
            t1 = sb.tile([BC, 256], F32, tag="t1")
            nc.vector.tensor_tensor(out=t1[:], in0=psBC[:, 0:256], in1=rz[:, 0:256], op=ALU.mult)
            nc.vector.tensor_tensor(out=t1[:], in0=t1[:], in1=psBC[:, 256:512], op=ALU.add)
            nn_ = sb.tile([BC, 256], F32, tag="nn")
            nc.scalar.activation(nn_[:], t1[:], AF.Tanh)
            nc.vector.tensor_tensor(out=t1[:], in0=h1[:], in1=nn_[:], op=ALU.subtract)
            nc.vector.tensor_tensor(out=t1[:], in0=t1[:], in1=rz[:, 256:512], op=ALU.mult)
            nc.vector.tensor_tensor(out=h1[:], in0=nn_[:], in1=t1[:], op=ALU.add)
            hb = sb.tile([BC, 256], BF16, tag="hb")
            nc.vector.tensor_copy(hb[:], h1[:])
            pT = ps2.tile([128, 256], BF16, tag="Tr")
            nc.tensor.transpose(pT[:, 0:128], hb[:, 0:128], ident[:])
            nc.tensor.transpose(pT[:, 128:256], hb[:, 128:256], ident[:])
            nc.vector.tensor_copy(agin[:], pT[:])

        def ag_h1(ag_pool, dr_pool, agin):
            """AllGather the [128,256] bf16 h1 slice -> fresh h1T [128, H]."""
            ag_i = dr_pool.tile([128, 256], BF16, tag="agi")
            ag_o = dr_pool.tile([512, 256], BF16, tag="ago")
            nc.sync.dma_start(out=ag_i[:], in_=agin[:])
            nc.gpsimd.collective_compute("AllGather", ALU.bypass, ins=[ag_i.opt()],
                                         outs=[ag_o.opt()], replica_groups=GROUPS)
            h1T_new = ag_pool.tile([128, H], BF16, tag="hT1")
            nc.sync.dma_start(out=h1T_new[:].rearrange("p (r tl b) -> p r tl b", r=4, tl=2),
                              in_=ag_o[:].rearrange("(r p) (tl b) -> p r tl b", r=4, tl=2))
            return h1T_new

        def load_xlo(t):
            t = t % T
            xlo = es.tile([7, 128], BF16, tag="xlo")
            nc.sync.dma_start(out=xlo[:], in_=XTlo_dr[:, t * 128:(t + 1) * 128])
            return xlo

        def l0_ih(t, xlo_tile):
            t = t % T
            return [(XThi[:, t * 128:(t + 1) * 128], wih0hi[:]), (xlo_tile[:], wih0lo[:])]

        # ---------------- phase C: encode ----------------
        h1T = ea.tile([128, H], BF16, tag="hT1")
        nc.vector.memset(h1T[:], 0.0)

        xlo = load_xlo(0)
        l0_full(es, eb, ep, ep2, l0_ih(0, xlo))
        for t in range(t_steps):
            agin = ea.tile([128, 256], BF16, tag="agin")
            l1_cell(es, ep, ep2, h1T, agin)
            h1T = ag_h1(ea, ed, agin)
            if t + 1 < t_steps:
                xlo = load_xlo(t + 1)
                l0_full(es, eb, ep, ep2, l0_ih(t + 1, xlo))

        # hold final h1T across the encode-pool teardown
        nc.vector.tensor_copy(h1T_hold[:], h1T[:])

        ea_cm.__exit__(None, None, None)
        ed_cm.__exit__(None, None, None)
        ep2_cm.__exit__(None, None, None)
        ep_cm.__exit__(None, None, None)
        es_cm.__exit__(None, None, None)
        eb_cm.__exit__(None, None, None)
        XTlo_dr_cm.__exit__(None, None, None)
        XP_cm.__exit__(None, None, None)

        # ---------------- phase D: rollout ----------------
        rp_cm = tc.tile_pool(name="rp", bufs=1)
        rp = rp_cm.__enter__()
        pre_k = [rp.tile([128, H], BF16, tag=f"pre_{k}", name=f"pre_{k}") for k in range(8)]
        for k in range(8):
            nc.sync.dma_start(out=pre_k[k][:], in_=pre_wT[k * 128:(k + 1) * 128, :])
        pre_b_sb = load(pre_b, pool=rp)
        spl1_k = [rp.tile([128, J * 128], BF16, tag=f"spl1_{k}", name=f"spl1_{k}") for k in range(8)]
        for k in range(8):
            nc.sync.dma_start(out=spl1_k[k][:], in_=spl1T[k * 128:(k + 1) * 128, :])
        spl1b_sb = load(spl1b, pool=rp)
        spl2_sb = load(spl2, pool=rp)
        spl2b_sb = load(spl2b, pool=rp)
        p6d = rp.tile([BC, J * 6], F32, tag="p6d")
        nc.sync.dma_start(out=p6d[:], in_=prev6d0[:])
        xTh = rp.tile([128, BC], BF16, tag="xTh")
        xTl = rp.tile([7, BC], BF16, tag="xTl")
        nc.sync.dma_start(out=xTh[:], in_=x0T[0:128, :])
        nc.sync.dma_start(out=xTl[:], in_=x0T[128:D, :])

        rb_cm = tc.tile_pool(name="rb", bufs=1)
        rbp = rb_cm.__enter__()
        rss_cm = tc.tile_pool(name="rss", bufs=2)
        rss = rss_cm.__enter__()
        rps_cm = tc.tile_pool(name="rps", bufs=1, space="PSUM")
        rps = rps_cm.__enter__()
        rps2_cm = tc.tile_pool(name="rps2", bufs=2, space="PSUM")
        rps2 = rps2_cm.__enter__()
        rdr_cm = tc.tile_pool(name="rdr", bufs=3, space="DRAM")
        rdr = rdr_cm.__enter__()
        rag_cm = tc.tile_pool(name="rag", bufs=2)
        rag = rag_cm.__enter__()

        h1T = rag.tile([128, H], BF16, tag="hT1")
        nc.vector.tensor_copy(h1T[:], h1T_hold[:])

        for t in range(pred_steps):
            l0_full(rss, rbp, rps, rps2, [(xTh[:], wih0hi[:]), (xTl[:], wih0lo[:])])
            agin = rag.tile([128, 256], BF16, tag="agin")
            l1_cell(rss, rps, rps2, h1T, agin)
            h1T = ag_h1(rag, rdr, agin)
            # pre + ctx + relu -> hidden [BC, H] (replicated)
            hid = rbp.tile([BC, H], F32, tag="t1f")
            for s in range(2):
                sl = slice(s * 512, (s + 1) * 512)
                pP = rps.tile([BC, 512], F32, tag="pA")
                for k in range(8):
                    nc.tensor.matmul(pP[:], h1T[:, k * 128:(k + 1) * 128], pre_k[k][:, sl],
                                     start=(k == 0), stop=False)
                nc.tensor.matmul(pP[:], ones_col[:, 0:BC], pre_b_sb[:, sl], start=False, stop=True)
                nc.scalar.activation(hid[:, sl], pP[:], AF.Relu)
            nc.vector.tensor_tensor(out=hid[:], in0=hid[:], in1=ctx_sb[:], op=ALU.add)
            hidb = rbp.tile([BC, H], BF16, tag="hidb")
            nc.vector.tensor_copy(hidb[:], hid[:])
            hidT = rbp.tile([128, H], BF16, tag="hidT")
            for k in range(8):
                pT = rps2.tile([128, 128], BF16, tag="Tr")
                nc.tensor.transpose(pT[:], hidb[:, k * 128:(k + 1) * 128], ident[:])
                nc.vector.tensor_copy(hidT[:, k * 128:(k + 1) * 128], pT[:])
            # spline layer 1 (resident weights, 512-aligned ranges): z1 [BC, J*128]
            z1 = rbp.tile([BC, J * 128], BF16, tag="z1")
            for s in range(4):
                c0 = s * 512
                c1 = min(c0 + 512, J * 128)
                pZ = rps.tile([BC, 512], F32, tag="pA")
                for k in range(8):
                    nc.tensor.matmul(pZ[:, 0:c1 - c0], hidT[:, k * 128:(k + 1) * 128],
                                     spl1_k[k][:, c0:c1], start=(k == 0), stop=False)
                nc.tensor.matmul(pZ[:, 0:c1 - c0], ones_col[:, 0:BC], spl1b_sb[:, c0:c1],
                                 start=False, stop=True)
                nc.scalar.activation(z1[:, c0:c1], pZ[:, 0:c1 - c0], AF.Relu)
            # spline layer 2 per joint
            pD = rps.tile([BC, J * 6], F32, tag="pC")
            for j in range(J):
                pT = rps2.tile([128, 128], BF16, tag="Tr")
                nc.tensor.transpose(pT[:], z1[:, j * 128:(j + 1) * 128], ident[:])
                z1T = rss.tile([128, BC], BF16, tag="z1T")
                nc.vector.tensor_copy(z1T[:], pT[:])
                nc.tensor.matmul(pD[:, j * 6:(j + 1) * 6], z1T[:], spl2_sb[:, j * 6:(j + 1) * 6],
                                 start=True, stop=False)
                nc.tensor.matmul(pD[:, j * 6:(j + 1) * 6], ones_col[:, 0:BC],
                                 spl2b_sb[:, j * 6:(j + 1) * 6], start=False, stop=True)
            nc.vector.tensor_tensor(out=p6d[:], in0=p6d[:], in1=pD[:], op=ALU.add)
            nc.sync.dma_start(out=out6d[t % PRED, :, :], in_=p6d[:])
            # rot6d -> R -> x_new -> transposed x for next step
            v6 = p6d[:].rearrange("p (j r w) -> p j r w", r=3, w=2)
            a1, a2 = v6[:, :, :, 0], v6[:, :, :, 1]
            t12 = rss.tile([BC, J * 3], F32, tag="t12")
            t12v = t12[:].rearrange("p (j c) -> p j c", c=3)
            n4 = rss.tile([BC, J], F32, tag="n4")
            r4_ = rss.tile([BC, J], F32, tag="r4_")
            B1 = rss.tile([BC, J * 3], F32, tag="B1")
            B1v = B1[:].rearrange("p (j c) -> p j c", c=3)
            B2 = rss.tile([BC, J * 3], F32, tag="B2")
            B2v = B2[:].rearrange("p (j c) -> p j c", c=3)
            B3 = rss.tile([BC, J * 3], F32, tag="B3")
            B3v = B3[:].rearrange("p (j c) -> p j c", c=3)

            def normize(src, dst):
                nc.vector.tensor_tensor(out=t12v, in0=src, in1=src, op=ALU.mult)
                nc.vector.reduce_sum(n4[:, :, None], t12v, axis=mybir.AxisListType.X)
                nc.scalar.activation(n4[:], n4[:], AF.Sqrt)
                nc.vector.tensor_scalar_max(n4[:], n4[:], 1e-12)
                nc.vector.reciprocal(r4_[:], n4[:])
                nc.vector.tensor_tensor(out=dst, in0=src,
                                        in1=r4_[:, :, None].broadcast_to([BC, J, 3]), op=ALU.mult)

            normize(a1, B1v)
            nc.vector.tensor_tensor(out=t12v, in0=B1v, in1=a2, op=ALU.mult)
            nc.vector.reduce_sum(n4[:, :, None], t12v, axis=mybir.AxisListType.X)
            nc.vector.tensor_tensor(out=t12v, in0=B1v,
                                    in1=n4[:, :, None].broadcast_to([BC, J, 3]), op=ALU.mult)
            a2o = rss.tile([BC, J * 3], F32, tag="a2o")
            a2ov = a2o[:].rearrange("p (j c) -> p j c", c=3)
            nc.vector.tensor_tensor(out=a2ov, in0=a2, in1=t12v, op=ALU.subtract)
            normize(a2ov, B2v)
            for c in range(3):
                u, v = (c + 1) % 3, (c + 2) % 3
                m1 = rss.tile([BC, J], F32, tag="m1")
                nc.vector.tensor_tensor(out=m1[:, :, None], in0=B1v[:, :, u:u + 1],
                                        in1=B2v[:, :, v:v + 1], op=ALU.mult)
                m2 = rss.tile([BC, J], F32, tag="m2")
                nc.vector.tensor_tensor(out=m2[:, :, None], in0=B1v[:, :, v:v + 1],
                                        in1=B2v[:, :, u:u + 1], op=ALU.mult)
                nc.vector.tensor_tensor(out=B3v[:, :, c:c + 1], in0=m1[:, :, None],
                                        in1=m2[:, :, None], op=ALU.subtract)
            xn = rss.tile([BC, J * 9], BF16, tag="xn")
            xnv = xn[:].rearrange("p (j r c) -> p j r c", r=3, c=3)
            nc.vector.tensor_copy(xnv[:, :, :, 0:1], B1v[:, :, :, None])
            nc.vector.tensor_copy(xnv[:, :, :, 1:2], B2v[:, :, :, None])
            nc.vector.tensor_copy(xnv[:, :, :, 2:3], B3v[:, :, :, None])
            pT = rps2.tile([128, 128], BF16, tag="Tr")
            nc.tensor.transpose(pT[:], xn[:, 0:128], ident[:])
            nc.vector.tensor_copy(xTh[:], pT[:])
            pT2 = rps2.tile([7, 128], BF16, tag="Tr")
            nc.tensor.transpose(pT2[:], xn[:, 128:D], ident[:])
            nc.vector.tensor_copy(xTl[:], pT2[:])

        rag_cm.__exit__(None, None, None)
        rdr_cm.__exit__(None, None, None)
        rps2_cm.__exit__(None, None, None)
        rps_cm.__exit__(None, None, None)
        rss_cm.__exit__(None, None, None)
        rb_cm.__exit__(None, None, None)
        rp_cm.__exit__(None, None, None)
        wq_cm.__exit__(None, None, None)
        wp_cm.__exit__(None, None, None)
        rs_cm.__exit__(None, None, None)
    nc.compile()
    return nc


# ---------------- host side ----------------
_cached = {}


class _SpmdRunner:
    def __init__(self, nc, n_cores):
        import jax
        from jax.sharding import Mesh, PartitionSpec
        from jax.experimental.shard_map import shard_map
        from concourse import bass2jax
        from concourse.bass2jax import _bass_exec_p, partition_id_tensor
        bass2jax.install_neuronx_cc_hook()
        self.jax = jax
        self.PartitionSpec = PartitionSpec
        self.n_cores = n_cores
        in_names, out_names, out_avals, zero_outs = [], [], [], []
        pname = nc.partition_id_tensor.name if nc.partition_id_tensor else None
        for alloc in nc.m.functions[0].allocations:
            if not isinstance(alloc, mybir.MemoryLocationSet):
                continue
            name = alloc.memorylocations[0].name
            if alloc.kind == "ExternalInput":
                if name != pname:
                    in_names.append(name)
            elif alloc.kind == "ExternalOutput":
                out_names.append(name)
                shape = tuple(alloc.tensor_shape)
                dtype = mybir.dt.np(alloc.dtype)
                out_avals.append(jax.core.ShapedArray(shape, dtype))
                zero_outs.append(np.zeros(shape, dtype))
        self.in_names, self.out_names = in_names, out_names
        self.out_avals, self.zero_outs = out_avals, zero_outs
        n_params, n_outs = len(in_names), len(out_names)
        all_in = in_names + out_names + ([pname] if pname else [])

        def _body(*args):
            operands = list(args)
            if pname is not None:
                operands.append(partition_id_tensor())
            return tuple(_bass_exec_p.bind(
                *operands, out_avals=tuple(out_avals), in_names=tuple(all_in),
                out_names=tuple(out_names), lowering_input_output_aliases=(),
                sim_require_finite=True, sim_require_nnan=True, nc=nc))

        devices = jax.devices()[:n_cores]
        self.mesh = Mesh(np.asarray(devices), ("core",))
        specs = (PartitionSpec("core"),) * (n_params + n_outs)
        self.fn = jax.jit(shard_map(_body, mesh=self.mesh, in_specs=specs,
                                    out_specs=(PartitionSpec("core"),) * n_outs,
                                    check_rep=False), keep_unused=True)

    def put(self, in_maps):
        import jax
        from jax.sharding import NamedSharding
        sh = NamedSharding(self.mesh, self.PartitionSpec("core"))
        args = []
        for name in self.in_names:
            arr = np.concatenate([np.asarray(m[name]) for m in in_maps], axis=0)
            args.append(jax.device_put(arr, sh))
        for z in self.zero_outs:
            args.append(jax.device_put(np.concatenate([z] * self.n_cores, axis=0), sh))
        return args

    def run(self, args):
        import jax
        outs = self.fn(*args)
        jax.block_until_ready(outs)
        return outs

    def results(self, outs):
        res = []
        for c in range(self.n_cores):
            d = {}
            for i, name in enumerate(self.out_names):
                d[name] = np.asarray(outs[i]).reshape(self.n_cores, *self.out_avals[i].shape)[c]
            res.append(d)
        return res


def get_runner(t_steps=T_STEPS, pred_steps=PRED_STEPS):
    key = (t_steps, pred_steps)
    if key not in _cached:
        nc = build_module(t_steps, pred_steps)
        _cached[key] = _SpmdRunner(nc, 8)
    return _cached[key]


def make_in_maps(inputs):
    poses = np.asarray(inputs["poses"], np.float32)
    freq_w, freq_b = inputs["freq_w"], inputs["freq_b"]
    attn_in_w, attn_in_b = inputs["attn_in_w"], inputs["attn_in_b"]
    attn_out_w, attn_out_b = inputs["attn_out_w"], inputs["attn_out_b"]
    Wf = (freq_w.T.astype(np.float64) @ attn_in_w.T.astype(np.float64)).astype(np.float32)
    bfull = (freq_b.astype(np.float64) @ attn_in_w.T.astype(np.float64)).astype(np.float32) + attn_in_b
    k_ = np.arange(FQ)[None, :]
    t_ = np.arange(T)[:, None]
    ct = np.cos(2 * np.pi * k_ * t_ / T).astype(np.float32)  # [T, FQ]

    x0 = poses[:, T - 1, :]
    R0 = x0.reshape(B, J, 3, 3)
    prev6d0 = np.concatenate([R0[..., 0], R0[..., 1]], axis=-1).reshape(B, J * 6)

    def hsl(w, l):  # w [.., 3072] -> L1 gate slice cols for lane l
        r = w[..., l * 256:(l + 1) * 256]
        z = w[..., 1024 + l * 256:1024 + (l + 1) * 256]
        n = w[..., 2048 + l * 256:2048 + (l + 1) * 256]
        return np.concatenate([r, z, n], axis=-1)

    def bsl(b1, b2, l):
        s = b1 + b2
        return (np.concatenate([s[l * 256:(l + 1) * 256],
                                s[1024 + l * 256:1024 + (l + 1) * 256]])[None],
                b1[2048 + l * 256:2048 + (l + 1) * 256][None],
                b2[2048 + l * 256:2048 + (l + 1) * 256][None])

    b0s = inputs["gru_bih0"] + inputs["gru_bhh0"]
    spl1T_f = _bf(np.concatenate([inputs["spl_w1"][j].T for j in range(J)], axis=1))
    spl1b_f = _bf(np.concatenate([inputs["spl_b1"][j] for j in range(J)])[None])
    spl2_f = _bf(np.concatenate([inputs["spl_w2"][j].T for j in range(J)], axis=1))
    spl2b_f = _bf(np.concatenate([inputs["spl_b2"][j] for j in range(J)])[None])

    in_maps = []
    for c in range(8):
        g, l = c // 4, c % 4
        bs = slice(g * BC, (g + 1) * BC)
        wfh = np.concatenate([Wf[:, l * 256:(l + 1) * 256],
                              Wf[:, 1024 + l * 256:1024 + (l + 1) * 256],
                              Wf[:, 2048 + l * 256:2048 + (l + 1) * 256]], axis=1)
        bfh = np.concatenate([bfull[l * 256:(l + 1) * 256],
                              bfull[1024 + l * 256:1024 + (l + 1) * 256],
                              bfull[2048 + l * 256:2048 + (l + 1) * 256]])[None]
        brz1_, bni1_, bnh1_ = bsl(inputs["gru_bih1"], inputs["gru_bhh1"], l)
        m = {
            "poses_b": _bf(poses[bs, :T, :]),
            "poses32": poses[bs, :T, :].astype(np.float32),
            "ct32": ct,
            "wf_qk32": wfh.astype(np.float32),
            "bf_qk32": bfh.astype(np.float32),
            "wout_h": attn_out_w[:, l * 256:(l + 1) * 256].T.astype(np.float32),
            "aob4": (attn_out_b / 4.0)[None].astype(np.float32),
            "wih0f": _bf(inputs["gru_wih0"].T),
            "whh0f": _bf(inputs["gru_whh0"].T),
            "b0rz": _bf(b0s[0:2 * H][None]),
            "b0ni": _bf(inputs["gru_bih0"][2 * H:3 * H][None]),
            "b0nh": _bf(inputs["gru_bhh0"][2 * H:3 * H][None]),
            "wih1T_s": _bf(hsl(inputs["gru_wih1"].T, l)),
            "whh1T_s": _bf(hsl(inputs["gru_whh1"].T, l)),
            "brz1": _bf(brz1_), "bni1": _bf(bni1_), "bnh1": _bf(bnh1_),
            "pre_wT": _bf(inputs["pre_w"].T), "pre_b_row": _bf(inputs["pre_b"][None]),
            "spl1T_f": spl1T_f, "spl1b_f": spl1b_f,
            "spl2_f": spl2_f, "spl2b_f": spl2b_f,
            "x0T": _bf(x0[bs].T),
            "prev6d0_s": prev6d0[bs].astype(np.float32),
        }
        in_maps.append(m)
    return in_maps


def assemble_output(res, pred_steps=PRED_STEPS):
    pred6d = np.zeros((B, pred_steps, J * 6), np.float32)
    for g in range(2):
        o = res[g * 4]["out6d"][:pred_steps]          # [pred, BC, J*6] from core 0 / 4
        pred6d[g * BC:(g + 1) * BC] = o.transpose(1, 0, 2)
    return pred6d


def kernel(**inputs):
    runner = get_runner()
    in_maps = make_in_maps(inputs)
    args = runner.put(in_maps)
    res = runner.results(runner.run(args))
    return assemble_output(res)
